# revision 48
# baseline (speedup 1.0000x reference)
"""Trainium2 Bass kernel for nn_Attention_13984413516503 (sparse_attention).

Sharding: 16 heads tensor-parallel over 8 NeuronCores (2 heads/core).
Per core: QKV projections for its heads, RoPE, two-softmax gated attention.
o_proj redistribution via AllToAll (token-sharded): each core ends with all
2048 head-dims for its 128-token slice per batch and computes out[tok, 2048
features] locally (full wo resident), so the collective moves 8x less data
than an AllGather of heads.

All shapes hardcoded for: B=2, S=1024, D=2048, H=16, HD=128, AL=10.
"""

import math

import numpy as np
import ml_dtypes

BF16 = ml_dtypes.bfloat16

B, S, D = 2, 1024, 2048
H, HD = 16, 128
AL = 10          # adapter length
MF = 10          # MAX_FEATS
NCORES = 8
HPC = H // NCORES          # heads per core = 2
TOK = B * S                # 2048
XTOK = AL + TOK            # 2058 (adapter ++ tokens)
ISC = 1.0 / math.sqrt(HD)  # 1/sqrt(128)

_BUILT = None
LAST_EXEC_NS = None
LAST_RES = None


def _build():
    import concourse.bass as bass
    import concourse.mybir as mybir
    import concourse.tile as tile
    from concourse import bacc

    dt = mybir.dt
    AF = mybir.ActivationFunctionType

    nc = bacc.Bacc(
        "TRN2", target_bir_lowering=False, debug=False, num_devices=NCORES
    )

    # ---- kernel I/O ----
    xa = nc.dram_tensor("xa", [D, XTOK], dt.bfloat16, kind="ExternalInput")
    wqkv = nc.dram_tensor("wqkv", [D, 6 * HD], dt.bfloat16, kind="ExternalInput")
    wod = nc.dram_tensor("wo", [D, D], dt.bfloat16, kind="ExternalInput")
    c2d = nc.dram_tensor("c2", [HD, TOK], dt.bfloat16, kind="ExternalInput")
    s2d = nc.dram_tensor("s2", [HD, TOK], dt.bfloat16, kind="ExternalInput")
    trid = nc.dram_tensor("tri", [HD, HD], dt.bfloat16, kind="ExternalInput")
    identd = nc.dram_tensor("ident", [HD, HD], dt.bfloat16, kind="ExternalInput")
    g2md = nc.dram_tensor("g2m", [HD, HPC * S], dt.bfloat16, kind="ExternalInput")
    g1cd = nc.dram_tensor("g1c", [AL, HPC], dt.float32, kind="ExternalInput")
    out_ext = nc.dram_tensor("out", [B * 128, D], dt.float32, kind="ExternalOutput")
    import os as _os
    _DBG = bool(_os.environ.get("KERNEL_DEBUG_DUMP"))
    if _DBG:
        dbg_at = nc.dram_tensor("dbg_at", [HPC * 128, TOK], dt.bfloat16, kind="ExternalOutput")
        dbg_b2c = nc.dram_tensor("dbg_b2c", [8 * HPC * HD, 128], dt.bfloat16, kind="ExternalOutput")
        dbg_a2o = nc.dram_tensor("dbg_a2o", [H * HD, 128], dt.bfloat16, kind="ExternalOutput")

    # internal DRAM for the collectives
    wupin = nc.dram_tensor("wupin", [8, 64], dt.bfloat16)
    wupout = nc.dram_tensor("wupout", [8, 64], dt.bfloat16)
    # A2A bounce: in per batch [8 shards][2 heads * 128 hd][128 tok]
    b2c = [nc.dram_tensor(f"a2ain{b}", [8, HPC * HD, 128], dt.bfloat16) for b in range(B)]
    # A2A out per batch: [2048 head-dims][128 tok]
    a2o = [nc.dram_tensor(f"a2aout{b}", [H * HD, 128], dt.bfloat16) for b in range(B)]
    RG = [list(range(NCORES))]

    KT16 = D // 128  # 16 contraction tiles for projections / o_proj

    with tile.TileContext(nc, num_cores=NCORES) as tc:
        import contextlib

        ctx = contextlib.ExitStack()
        with ctx:
            # PSUM banks (8): psum3 4 + psumpv 2 + psumtp 2.  psumtp's "tp"
            # slot is shared by proj-phase transposes ([128,128]bf16) and
            # attention denominators ([64,512]f32) — disjoint phases.
            psum3 = ctx.enter_context(tc.tile_pool(name="psum3", bufs=4, space="PSUM"))
            psumpv = ctx.enter_context(tc.tile_pool(name="psumpv", bufs=2, space="PSUM"))
            psumtp = ctx.enter_context(tc.tile_pool(name="psumtp", bufs=2, space="PSUM"))
            consts = ctx.enter_context(tc.tile_pool(name="consts", bufs=1))
            work = ctx.enter_context(tc.tile_pool(name="work", bufs=1))

            # ---- persistent constants (DMAs issued later, after critical loads) ----
            c2 = consts.tile([HD, TOK], dt.bfloat16, tag="c2")
            s2 = consts.tile([HD, TOK], dt.bfloat16, tag="s2")
            tri = consts.tile([HD, HD], dt.bfloat16, tag="tri")
            ident = consts.tile([HD, HD], dt.bfloat16, tag="ident")
            g2m = consts.tile([HD, HPC * S], dt.bfloat16, tag="g2m")
            g1c = consts.tile([AL, HPC], dt.float32, tag="g1c")
            ocol = consts.tile([128, 1], dt.bfloat16, tag="ocol")
            nc.vector.memset(ocol[:], 1.0)
            orow = consts.tile([1, 128], dt.bfloat16, tag="orow")
            nc.vector.memset(orow[:], 1.0)
            wrm = consts.tile([128, 64], dt.bfloat16, tag="wrm")
            nc.vector.memset(wrm[:], 0.0)

            # PE pre-warm: sustained tiny matmuls during the startup DMA window
            # flip the HAM clock gate to 8/8 before the first real matmul.
            for _ in range(36):
                wps = psum3.tile([128, 512], dt.float32, tag="mm")
                nc.tensor.matmul(wps[:64, :64], wrm[:], wrm[:], start=True, stop=True)

            # proj destinations: QR, QI, KR, KI, V0, V1 (paired-head layout)
            pdst = [
                work.tile([128, XTOK], dt.bfloat16, tag=f"pd{m}", name=f"pd{m}") for m in range(6)
            ]
            QR, QI, KR, KI = pdst[0], pdst[1], pdst[2], pdst[3]
            VT = [pdst[4], pdst[5]]

            QT = [work.tile([128, XTOK], dt.bfloat16, tag=f"qt{h}", name=f"qt{h}") for h in range(HPC)]
            KTt = [work.tile([128, XTOK], dt.bfloat16, tag=f"kt{h}", name=f"kt{h}") for h in range(HPC)]
            vtr = [work.tile([128, B * 8, 128], dt.bfloat16, tag=f"vtr{h}", name=f"vtr{h}") for h in range(HPC)]
            avt = [work.tile([AL, 128], dt.bfloat16, tag=f"avt{h}", name=f"avt{h}") for h in range(HPC)]

            attnT = [work.tile([128, TOK], dt.bfloat16, tag=f"at{h}", name=f"at{h}") for h in range(HPC)]
            epool = ctx.enter_context(tc.tile_pool(name="epool", bufs=16))
            espool = ctx.enter_context(tc.tile_pool(name="espool", bufs=8))
            eapool = ctx.enter_context(tc.tile_pool(name="eapool", bufs=2))
            npool = ctx.enter_context(tc.tile_pool(name="npool", bufs=2))

            # ---- attention stages (software-pipelined: A = scores/exp/pv/denoms,
            #      B = adapter fold + normalize; B emitted ~one proj-chunk later) ----
            def attn_S(b, h, qc):
                """scores -> exp (adapter + video tiles)."""
                base_k = AL + S * b
                qcol = base_k + 512 * qc
                nt = 4 * qc + 4
                # adapter scores -> Ea
                sa = psum3.tile([128, 512], dt.float32, tag="mm")
                nc.tensor.matmul(
                    sa[:AL, :], KTt[h][:, 0:AL], QT[h][:, qcol : qcol + 512],
                    start=True, stop=True,
                )
                ea = eapool.tile([AL, 512], dt.bfloat16, tag="ea")
                nc.scalar.activation(ea[:], sa[:AL, :], AF.Exp, scale=ISC)
                # video scores -> Ev tiles (sub-range on diagonal tiles)
                evs = []
                for t in range(nt):
                    j = t - 4 * qc  # diagonal block index within this q-chunk
                    lo = 128 * j if j > 0 else 0
                    sp = psum3.tile([128, 512], dt.float32, tag="mm")
                    nc.tensor.matmul(
                        sp[:, lo:512],
                        KTt[h][:, base_k + 128 * t : base_k + 128 * (t + 1)],
                        QT[h][:, qcol + lo : qcol + 512],
                        start=True, stop=True,
                    )
                    ev = epool.tile([128, 512], dt.bfloat16, tag="ev")
                    nc.scalar.activation(ev[:, lo:512], sp[:, lo:512], AF.Exp, scale=ISC)
                    if j >= 0:
                        if j > 0:
                            nc.vector.memset(ev[:, 0:lo], 0.0)
                        nc.vector.tensor_mul(
                            ev[:, 128 * j : 128 * (j + 1)],
                            ev[:, 128 * j : 128 * (j + 1)],
                            tri[:],
                        )
                    if t == 0:
                        nc.vector.tensor_mul(
                            ev[:], ev[:], g2m[:, S * h + 512 * qc : S * h + 512 * (qc + 1)]
                        )
                    evs.append(ev)
                return ea, evs

            def attn_P(b, h, qc, ea, evs):
                """pv(video) -> da/dv -> norm scalars."""
                # video PV accumulation (video part only; adapter folded in attn_B)
                pv = psumpv.tile([128, 512], dt.float32, tag="pv")
                for t, ev in enumerate(evs):
                    j = t - 4 * qc
                    lo = 128 * j if j > 0 else 0
                    nc.tensor.matmul(
                        pv[:, lo:512], vtr[h][:, 8 * b + t, :], ev[:, lo:512],
                        start=(t == 0), stop=False, skip_group_check=True,
                    )
                # denominators: Da (adapter) and Dv (video), both at partition 0
                da = psumtp.tile([1, 512], dt.float32, tag="tp")
                nc.tensor.matmul(da[:], ocol[0:AL, :], ea[:], start=True, stop=True)
                # Dv: DVE pairwise add-tree over Ev tiles, single ones-MM at the end
                lvl = list(evs)
                while len(lvl) > 1:
                    nxt = []
                    for i in range(0, len(lvl) - 1, 2):
                        sm = espool.tile([128, 512], dt.bfloat16, tag="evsum")
                        nc.vector.tensor_add(sm[:], lvl[i][:], lvl[i + 1][:])
                        nxt.append(sm)
                    if len(lvl) % 2:
                        nxt.append(lvl[-1])
                    lvl = nxt
                dv = psumtp.tile([1, 512], dt.float32, tag="tp")
                nc.tensor.matmul(dv[:], ocol[:], lvl[0][:], start=True, stop=True)
                # normalization scalars (DVE; off the PE stream — the PE-side
                # broadcasts happen in attn_B, a proj-chunk later, when these
                # row vectors are long since ready)
                raf = npool.tile([1, 512], dt.float32, tag="nf")
                nc.vector.reciprocal_approx_fast(raf[:], da[:])
                rr16 = npool.tile([1, 512], dt.bfloat16, tag="nf2")
                nc.vector.tensor_mul(rr16[:], raf[:], dv[:])
                rvf = npool.tile([1, 512], dt.float32, tag="nf3")
                nc.vector.reciprocal_approx_fast(rvf[:], dv[:])
                rv16 = npool.tile([1, 512], dt.bfloat16, tag="nf4")
                nc.vector.tensor_copy(rv16[:], rvf[:])
                return pv, ea, rr16, rv16

            def attn_A(b, h, qc):
                ea, evs = attn_S(b, h, qc)
                return attn_P(b, h, qc, ea, evs)

            def attn_B(b, h, qc, st):
                """broadcasts, adapter fold into pv, normalize by 1/Dv."""
                pv, ea, rr16, rv16 = st
                eas = psum3.tile([128, 512], dt.float32, tag="mm")
                nc.tensor.matmul(eas[:AL, :], orow[0:1, 0:AL], rr16[:], start=True, stop=True)
                rvps = psum3.tile([128, 512], dt.float32, tag="mm")
                nc.tensor.matmul(rvps[:], orow[0:1, :], rv16[:], start=True, stop=True)
                ea2 = eapool.tile([AL, 512], dt.bfloat16, tag="ea2")
                nc.vector.tensor_mul(ea2[:], ea[:], eas[:AL, :])
                rvb = npool.tile([128, 512], dt.bfloat16, tag="rvb")
                nc.scalar.copy(rvb[:], rvps[:])
                nc.tensor.matmul(
                    pv[:], avt[h][:], ea2[:], start=False, stop=True,
                    skip_group_check=True,
                )
                nc.vector.tensor_mul(
                    attnT[h][:, S * b + 512 * qc : S * b + 512 * (qc + 1)],
                    pv[:], rvb[:],
                )

            def a2a_batch(b):
                # bounce on the scalar (Activation) HWDGE queue: keeps it off
                # the sync queue (head-of-line: the 8MB wo load); exps for this
                # batch have drained by the time these are emitted
                for h in range(HPC):
                    nc.scalar.dma_start(
                        b2c[b][:, 128 * h : 128 * (h + 1), :].rearrange("j p c -> p j c"),
                        attnT[h][:, S * b : S * (b + 1)].rearrange("p (j c) -> p j c", j=8),
                    )
                nc.gpsimd.collective_compute(
                    "AllToAll",
                    bass.mybir.AluOpType.bypass,
                    replica_groups=RG,
                    ins=[b2c[b][:, :, :].opt()],
                    outs=[a2o[b][:, :].opt()],
                )

            # warmup collective: absorb ncfw/channel startup cost during load
            nc.gpsimd.collective_compute(
                "AllToAll", bass.mybir.AluOpType.bypass, replica_groups=RG,
                ins=[wupin[:, :].opt()], outs=[wupout[:, :].opt()],
            )

            with tc.tile_pool(name="p1", bufs=1) as p1pool, tc.tile_pool(name="rope", bufs=2) as rp:
                wq_k = [p1pool.tile([128, 6 * HD], dt.bfloat16, tag=f"wq{k}", name=f"wq{k}") for k in range(KT16)]
                # xa tiles per (cchunk, k): col ranges [0:522),[522:1034),[1034:1546),[1546:2058)
                ccol = [(0, 522), (522, 512), (1034, 512), (1546, 512)]
                xs = [
                    [p1pool.tile([128, 522], dt.bfloat16, tag=f"xa{min(ci, 3) if ci < 3 else 0}_{k}", name=f"xa{ci}_{k}") for k in range(KT16)]
                    for ci in range(4)
                ]
                # two HWDGE queues (sync=SP, scalar=Activation): split the
                # startup bulk by k parity so both queues stream in parallel.
                def deng(k):
                    return nc.sync if k % 2 == 0 else nc.scalar

                # first m-groups (KR/KI) need wq cols 256:512 + chunk 0 only
                for k in range(KT16):
                    deng(k).dma_start(wq_k[k][:, 256:512], wqkv[128 * k : 128 * (k + 1), 256:512])
                    x0, xw = ccol[0]
                    deng(k).dma_start(xs[0][k][:, :xw], xa[128 * k : 128 * (k + 1), x0 : x0 + xw])
                for k in range(KT16):
                    deng(k).dma_start(wq_k[k][:, 0:256], wqkv[128 * k : 128 * (k + 1), 0:256])
                    deng(k).dma_start(wq_k[k][:, 512:768], wqkv[128 * k : 128 * (k + 1), 512:768])
                # consts after the chunk-0-critical loads
                nc.sync.dma_start(c2[:], c2d[:, :])
                nc.sync.dma_start(s2[:], s2d[:, :])
                nc.sync.dma_start(ident[:], identd[:, :])
                nc.sync.dma_start(tri[:], trid[:, :])
                nc.sync.dma_start(g2m[:], g2md[:, :])
                nc.sync.dma_start(g1c[:], g1cd[:, :])
                for k in range(KT16):
                    x0, xw = ccol[1]
                    deng(k).dma_start(xs[1][k][:, :xw], xa[128 * k : 128 * (k + 1), x0 : x0 + xw])

                def load_xs(ci):
                    # late chunks go on sync only, emitted just-in-time so they
                    # don't head-of-line-block rope writes / attention exps
                    x0, xw = ccol[ci]
                    for k in range(KT16):
                        nc.sync.dma_start(xs[ci][k][:, :xw], xa[128 * k : 128 * (k + 1), x0 : x0 + xw])

                def rope_chunk(xr, xi, tc0, c0):
                    # tc0: token col offset in [0,2048); c0 = AL + tc0 (col in pdst)
                    cs = c2[:, tc0 : tc0 + 512]
                    sn = s2[:, tc0 : tc0 + 512]
                    a = rp.tile([128, 512], dt.bfloat16, tag="ra")
                    b_ = rp.tile([128, 512], dt.bfloat16, tag="rb")
                    nc.vector.tensor_mul(a[:], xr[:, c0 : c0 + 512], cs)
                    nc.vector.tensor_mul(b_[:], xi[:, c0 : c0 + 512], sn)
                    ro = rp.tile([128, 512], dt.bfloat16, tag="rro")
                    nc.vector.tensor_sub(ro[:], a[:], b_[:])
                    c_ = rp.tile([128, 512], dt.bfloat16, tag="rc")
                    d_ = rp.tile([128, 512], dt.bfloat16, tag="rd")
                    nc.vector.tensor_mul(c_[:], xr[:, c0 : c0 + 512], sn)
                    nc.vector.tensor_mul(d_[:], xi[:, c0 : c0 + 512], cs)
                    io = rp.tile([128, 512], dt.bfloat16, tag="rio")
                    nc.vector.tensor_add(io[:], c_[:], d_[:])
                    return ro, io

                def post_m(m, ci):
                    c0 = AL + 512 * ci
                    tc0 = 512 * ci
                    if m == 3:   # KR+KI done for this chunk
                        ro, io = rope_chunk(KR, KI, tc0, c0)
                        for h in range(HPC):
                            hs = slice(64 * h, 64 * h + 64)
                            nc.sync.dma_start(KTt[h][0:64, c0 : c0 + 512], ro[hs, :])
                            nc.sync.dma_start(KTt[h][64:128, c0 : c0 + 512], io[hs, :])
                    elif m == 1:  # QR+QI done
                        ro, io = rope_chunk(QR, QI, tc0, c0)
                        for h in range(HPC):
                            hs = slice(64 * h, 64 * h + 64)
                            nc.sync.dma_start(QT[h][0:64, c0 : c0 + 512], ro[hs, :])
                            nc.sync.dma_start(QT[h][64:128, c0 : c0 + 512], io[hs, :])
                    elif m >= 4:  # V chunk ready -> PE transposes into vtr
                        h = m - 4
                        bb, thalf = ci // 2, 4 * (ci % 2)
                        for tt in range(4):
                            tp = psumtp.tile([128, 128], dt.bfloat16, tag="tp")
                            nc.tensor.transpose(tp[:], VT[h][:, c0 + 128 * tt : c0 + 128 * (tt + 1)], ident[:])
                            nc.scalar.copy(vtr[h][:, 8 * bb + thalf + tt, :], tp[:])

                def chunk_part(ci, ms):
                    xoff0 = AL if ci == 0 else 0
                    for m in ms:
                        psa = psum3.tile([128, 512], dt.float32, tag="mm")
                        for k in range(KT16):
                            nc.tensor.matmul(psa[:], wq_k[k][:, 128 * m : 128 * (m + 1)],
                                             xs[ci][k][:, xoff0 : xoff0 + 512],
                                             start=(k == 0), stop=(k == KT16 - 1))
                        nc.vector.tensor_copy(pdst[m][:, AL + 512 * ci : AL + 512 * (ci + 1)], psa[:])
                        post_m(m, ci)

                def solo_chunk(ci):
                    chunk_part(ci, (2, 3, 0, 1, 4, 5))

                def adapter_cols(ms):
                    for m in ms:
                        psa = psum3.tile([128, 512], dt.float32, tag="mm")
                        for k in range(KT16):
                            nc.tensor.matmul(
                                psa[:, :AL],
                                wq_k[k][:, 128 * m : 128 * (m + 1)],
                                xs[0][k][:, 0:AL],
                                start=(k == 0), stop=(k == KT16 - 1),
                            )
                        nc.vector.tensor_copy(pdst[m][:, 0:AL], psa[:, :AL])

                # adapter K in the startup DMA window (its wave-1 inputs land
                # tile by tile; the LDW-bound matmuls pace the arrivals)
                adapter_cols((2, 3))
                for h in range(HPC):
                    hs = slice(64 * h, 64 * h + 64)
                    nc.sync.dma_start(KTt[h][0:64, 0:AL], KR[hs, 0:AL])
                    nc.sync.dma_start(KTt[h][64:128, 0:AL], KI[hs, 0:AL])
                solo_chunk(0)
                adapter_cols((4, 5))
                for h in range(HPC):
                    # adapter V: transpose + tanh(gate1) scale
                    tp = psumtp.tile([128, 128], dt.bfloat16, tag="tp")
                    nc.tensor.transpose(tp[:AL, :], VT[h][:, 0:AL], ident[:])
                    nc.vector.tensor_scalar_mul(avt[h][:], tp[:AL, :], g1c[:, h : h + 1])

                solo_chunk(1)
                load_xs(2)
                stA0 = [attn_A(0, h, 0) for h in range(HPC)]
                solo_chunk(2)
                load_xs(3)
                for h in range(HPC):
                    attn_B(0, h, 0, stA0[h])
                stA1 = [attn_A(0, h, 1) for h in range(HPC)]
                # chunk 3 split in half around the b0 -> b1 attention handoff:
                # b0's last folds + A2A(b0) trigger mid-chunk, and b1/qc0's
                # exp chain hides under the second half's matmuls
                chunk_part(3, (2, 3, 0))
                for h in range(HPC):
                    attn_B(0, h, 1, stA1[h])
                a2a_batch(0)
                stB0 = [attn_A(1, h, 0) for h in range(HPC)]
                chunk_part(3, (1,))
                for h in range(HPC):
                    attn_B(1, h, 0, stB0[h])
                # b1/qc1 scores+exp before the last V m-groups: the 15us exp
                # chain resolves under those matmuls instead of on the tail
                sS1 = [attn_S(1, h, 1) for h in range(HPC)]
                chunk_part(3, (4, 5))

            # projections done: p1/rope freed; stream full wo on both spare
            # queues (sync + gpsimd SWDGE) while batch-1 attention finishes
            wopool = ctx.enter_context(tc.tile_pool(name="wopool", bufs=1))
            wo_sb = wopool.tile([128, KT16, D], dt.bfloat16, tag="wo")

            def wo_slice(fc):
                return (
                    wo_sb[:, :, 512 * fc : 512 * (fc + 1)],
                    wod[:, 512 * fc : 512 * (fc + 1)].rearrange("(k p) c -> p k c", p=128),
                )

            ogp = ctx.enter_context(tc.tile_pool(name="ogp", bufs=2))
            aob0 = ogp.tile([128, KT16, 128], dt.bfloat16, tag="aob")
            aob1 = ogp.tile([128, KT16, 128], dt.bfloat16, tag="aob")
            # queue interleave tuned to o_proj's consumption order:
            # sync carries fc0, the A2A results, then fc2; gpsimd (SWDGE)
            # carries fc1 and fc3 in parallel.
            nc.sync.dma_start(*wo_slice(0))
            nc.gpsimd.dma_start(*wo_slice(1))
            nc.sync.dma_start(aob0[:], a2o[0][:, :].rearrange("(k p) c -> p k c", p=128))
            nc.sync.dma_start(*wo_slice(2))
            nc.gpsimd.dma_start(*wo_slice(3))

            def oproj_fc(b, fc, aob):
                ps = psum3.tile([128, 512], dt.float32, tag="mm")
                for k in range(KT16):
                    nc.tensor.matmul(
                        ps[:], aob[:, k, :], wo_sb[:, k, 512 * fc : 512 * (fc + 1)],
                        start=(k == 0), stop=(k == KT16 - 1),
                    )
                osb = ogp.tile([128, 512], dt.float32, tag="osb")
                nc.scalar.copy(osb[:], ps[:])
                nc.scalar.dma_start(
                    out_ext[128 * b : 128 * (b + 1), 512 * fc : 512 * (fc + 1)],
                    osb[:],
                )

            stB1 = [attn_P(1, h, 1, *sS1[h]) for h in range(HPC)]
            for h in range(HPC):
                attn_B(1, h, 1, stB1[h])
            a2a_batch(1)
            nc.sync.dma_start(aob1[:], a2o[1][:, :].rearrange("(k p) c -> p k c", p=128))
            # all of o_proj b0 sits after the A2A(b1) trigger: ~17us of PE
            # work covering the collective's peer-sync + data phases, and
            # keeping the clock warm for o_proj b1
            for fc in range(4):
                oproj_fc(0, fc, aob0)
            # clock-keepalive while the A2A(b1) result lands: dependency-free
            # tiny matmuls stop the HAM gate from dropping to half rate
            for _ in range(40):
                wps = psum3.tile([128, 512], dt.float32, tag="mm")
                nc.tensor.matmul(wps[:64, :64], wrm[:], wrm[:], start=True, stop=True)
            for fc in range(4):
                oproj_fc(1, fc, aob1)
            if _DBG:
                for h in range(HPC):
                    nc.sync.dma_start(dbg_at[128 * h : 128 * (h + 1), :], attnT[h][:, :])
                nc.sync.dma_start(dbg_b2c[:, :], b2c[0][:, :, :])
                nc.sync.dma_start(dbg_a2o[:, :], a2o[0][:, :])

    nc.finalize()
    return nc


def _host_prep(inputs):
    """Build the 8 per-core input maps from full inputs."""
    x = np.asarray(inputs["x"], np.float32)
    adapter = np.asarray(inputs["adapter"], np.float32)
    wq = np.asarray(inputs["wq"], np.float32)
    wk = np.asarray(inputs["wk"], np.float32)
    wv = np.asarray(inputs["wv"], np.float32)
    wo = np.asarray(inputs["wo"], np.float32)
    g1 = np.asarray(inputs["gate1"], np.float32).reshape(H)
    g2 = np.asarray(inputs["gate2"], np.float32).reshape(H)
    fc = np.asarray(inputs["freqs_cos"], np.float32)  # [S, 64]
    fs = np.asarray(inputs["freqs_sin"], np.float32)
    vs = int(inputs["video_start"])
    assert vs + MF <= 128, "gate2 block must stay in kt tile 0"

    # xa: [D, 10+2048] = adapter^T ++ x^T (bf16)
    xt = x.reshape(TOK, D).T
    at = adapter.reshape(AL, D).T
    xa = np.concatenate([at, xt], axis=1).astype(BF16)

    # RoPE split permutation per head: even dims then odd dims
    ev = np.arange(0, HD, 2)
    od = np.arange(1, HD, 2)

    # c2/s2: [128, 2048]; rows 0-63 for head h0's pairs, 64-127 for h1's pairs
    cosT = np.tile(fc.T, (1, B))  # [64, 2048]
    sinT = np.tile(fs.T, (1, B))
    c2 = np.vstack([cosT, cosT]).astype(BF16)
    s2 = np.vstack([sinT, sinT]).astype(BF16)

    tri = np.triu(np.ones((HD, HD), np.float32)).astype(BF16)
    ident = np.eye(HD, dtype=np.float32).astype(BF16)
    wot = wo.T.astype(BF16)  # [D(in=head dims), D(out features)], full per core

    in_maps = []
    for c in range(NCORES):
        hs = [HPC * c + i for i in range(HPC)]  # global head ids
        # paired-head m-tiles: QR=[h0_even,h1_even], QI=[h0_odd,h1_odd], same for K; V=[h0],[h1]
        def rows(w, h):  # weight rows for head h -> [128, D]
            return w[HD * h : HD * (h + 1), :]

        qr = np.vstack([rows(wq, hs[0])[ev], rows(wq, hs[1])[ev]])
        qi = np.vstack([rows(wq, hs[0])[od], rows(wq, hs[1])[od]])
        kr = np.vstack([rows(wk, hs[0])[ev], rows(wk, hs[1])[ev]])
        ki = np.vstack([rows(wk, hs[0])[od], rows(wk, hs[1])[od]])
        v0 = rows(wv, hs[0])
        v1 = rows(wv, hs[1])
        wqkv = np.concatenate([m.T for m in (qr, qi, kr, ki, v0, v1)], axis=1).astype(BF16)

        g2mat = np.ones((HD, HPC * S), np.float32)
        for i, h in enumerate(hs):
            blk = np.ones((HD, S), np.float32)
            blk[vs : vs + MF, vs + MF :] = math.exp(g2[h])
            g2mat[:, S * i : S * (i + 1)] = blk
        g2mat = g2mat.astype(BF16)

        g1cm = np.empty((AL, HPC), np.float32)
        for i, h in enumerate(hs):
            g1cm[:, i] = math.tanh(g1[h])

        in_maps.append(
            {
                "xa": xa, "wqkv": wqkv, "wo": wot, "c2": c2, "s2": s2,
                "tri": tri, "ident": ident, "g2m": g2mat, "g1c": g1cm,
            }
        )
    return in_maps


def _ensure_ntff_hook():
    import sys, types
    if "antenv.axon_hooks" in sys.modules:
        return
    try:
        from trn_agent_boot.trn_boot import _ntff_profile_via_ctypes
        hook = _ntff_profile_via_ctypes("/opt/axon/libaxon_pjrt.so")
        mod = types.ModuleType("antenv.axon_hooks")
        mod.get_axon_ntff_profile_hook = lambda: hook
        mod.set_axon_ntff_profile_hook = lambda h: None
        sys.modules["antenv.axon_hooks"] = mod
    except Exception:
        pass


def kernel(**inputs):
    global _BUILT, LAST_EXEC_NS, LAST_RES
    import os
    from concourse.bass_utils import run_bass_kernel_spmd

    if _BUILT is None:
        _BUILT = _build()
    nc = _BUILT
    in_maps = _host_prep(inputs)
    trace = bool(os.environ.get("KERNEL_TRACE"))
    if trace:
        _ensure_ntff_hook()
    res = run_bass_kernel_spmd(
        nc, in_maps, core_ids=list(range(NCORES)), trace=trace
    )
    LAST_EXEC_NS = res.exec_time_ns
    LAST_RES = res
    outs = [np.asarray(r["out"], np.float32) for r in res.results]
    # out_c: [B*128, 2048] = tokens (128 per batch) x features
    full = np.empty((B, S, D), np.float32)
    for i, o in enumerate(outs):
        full[0, 128 * i : 128 * (i + 1), :] = o[0:128]
        full[1, 128 * i : 128 * (i + 1), :] = o[128:256]
    return full
